# revision 16
# baseline (speedup 1.0000x reference)
"""ATNAggregation2d Trainium2 kernel (8 NeuronCores, data-parallel over B*H*W).

Math (per pixel n, M=8 processes, C=64 channels), derived from the reference:
    V_m   = c_w x_m + c_b
    Q     = wq_w mean_m(V_m) + wq_b
    K_m   = wk_w V_m + wk_b
    A_m   = wa_w V_m + wa_b
    s_m   = (Q . K_m)/8 ;  alpha = softmax_m(s) ;  z = sum_m alpha_m A_m

Everything is linear in x, so fuse on the host:
    Wk' = wk_w c_w ; Wq' = wq_w c_w ; Wa' = wa_w c_w
    bq' = wq_w c_b + wq_b ; ba' = wa_w c_b + wa_b
    K_m's bias is constant across m -> cancels in softmax.
    s_m = Qt . (Wk' x_m)  with Qt = Q/8 = Wq' xsum/64 + bq'/8
        = G . x_m         with G = Ws xsum + bs,
          Ws = Wk'^T Wq'/64, bs = Wk'^T bq'/8
    z   = (sum_m e_m A'_m)/(sum_m e_m) + ba' , e_m = exp(s_m), A'_m = Wa' x_m
(no max-subtraction needed: |s| << 1 for these inputs/scales).

On-chip layout: C on partitions, pixels on the free dim; the 8 processes are
packed as 4 pairs stacked into 128 partitions (pair j = processes 2j, 2j+1 at
partitions 0:64 / 64:128). Per-pixel partition reductions (score dot-products,
softmax denominator, the weighted sums) are done with small matmuls.
"""

import sys

for _p in ("/opt/trn_rl_repo", "/root/.axon_site/_ro/trn_rl_repo"):
    if _p not in sys.path:
        sys.path.append(_p)

import numpy as np
from ml_dtypes import bfloat16 as ml_bf16

import concourse.bass as bass
import concourse.tile as tile
from concourse import mybir
from concourse import bass_utils

M, B, C, H, W = 8, 2, 64, 96, 96
HW = H * W
N_CORES = 8
PIX_TOTAL = B * HW                 # 18432
PIX_CORE = PIX_TOTAL // N_CORES    # 2304 contiguous pixels of flat (B, H*W)
NPAIR = M // 2                     # 4 stacked process-pairs
TILE_NS = [512, 512, 512, 512, 256]  # per-core pixel tiles (sum = 2304)

FP32 = mybir.dt.float32


def _r(ap):
    """Bitcast an fp32 AP to float32r: single-pass PE matmul (4x faster than
    the fp32 two-half-pass path) at TF32-ish multiply precision."""
    return ap.bitcast(mybir.dt.float32r)


def _split_multi_waits(nc):
    """This walrus build accepts only ONE sync-wait command per instruction.
    Move extra on_wait entries onto Drain instructions inserted just before
    the owning instruction (same engine, program order preserved)."""
    for f in nc.m.functions:
        for bb in f.blocks:
            changed = False
            new = []
            for inst in bb.instructions:
                si = inst.sync_info
                if si is not None and si.on_wait and len(si.on_wait) > 1:
                    waits = list(si.on_wait)
                    for w in waits[:-1]:
                        d = mybir.InstNoOp(
                            name=nc.get_next_instruction_name(), ins=[], outs=[]
                        )
                        d.engine = inst.engine
                        d.sync_info = mybir.SyncInfo(on_wait=[w], on_update=[])
                        new.append(d)
                    inst.sync_info = mybir.SyncInfo(
                        on_wait=[waits[-1]], on_update=list(si.on_update)
                    )
                    changed = True
                new.append(inst)
            if changed:
                bb.instructions = new


def _fuse_weights(c_w, c_b, wq_w, wq_b, wk_w, wk_b, wa_w, wa_b):
    f8 = np.float64
    c_w, c_b = c_w.astype(f8), c_b.astype(f8)
    wq_w, wq_b = wq_w.astype(f8), wq_b.astype(f8)
    wk_w = wk_w.astype(f8)
    wa_w, wa_b = wa_w.astype(f8), wa_b.astype(f8)

    Wk = wk_w @ c_w
    Wq = wq_w @ c_w
    Wa = wa_w @ c_w
    bq = wq_w @ c_b + wq_b.astype(f8)
    ba = wa_w @ c_b + wa_b

    Ws = (Wk.T @ Wq) / 64.0
    bs = (Wk.T @ bq) / 8.0

    consts = {
        # G accumulation over 4 stacked pairs: out/in both [2x64] stacked.
        "Wg2": np.tile(Ws.T, (2, 2)).astype(ml_bf16),             # [128,128]
        "bs2": np.tile(bs, 2).reshape(128, 1).astype(np.float32),  # [128,1]
        # scores: per-pair partition-half dot products -> rows 32j, 32j+1
        # (bf16 matmul: fp32r would need 64-aligned dst partitions).
        "ones_sc": np.kron(np.eye(2), np.ones((64, 1))).astype(ml_bf16),  # [128,2]
        "Ebc": None,                                               # [128,128] below
        # softmax denominator: rows 32j,32j+1 weighted 1, all others 0.
        "onesD": None,                                             # [128,64] below
        # weighted-A accumulation, folds the two stacked halves.
        "WuT2": np.tile(Wa.T, (2, 1)).astype(ml_bf16),             # [128,64]
        "baD": None,                                               # [128,64] below
    }
    ebc = np.zeros((128, 128), dtype=ml_bf16)
    onesd = np.zeros((128, 64), dtype=ml_bf16)
    for j in range(4):
        for r in range(2):
            ebc[32 * j + r, 64 * r : 64 * r + 64] = 1.0
            onesd[32 * j + r, :] = 1.0
    consts["Ebc"] = ebc
    consts["onesD"] = onesd
    # z = (sum_m e_m A'_m + ba * sum_m e_m)/D  ==  U/D + ba: fold the output
    # bias into the U accumulation as an extra matmul over e_sb.
    bad = np.zeros((128, 64), dtype=ml_bf16)
    for j in range(4):
        for r in range(2):
            bad[32 * j + r, :] = ba.astype(np.float32)
    consts["baD"] = bad
    return consts


def _build_program():
    nc = bass.Bass()
    BF16 = mybir.dt.bfloat16
    xin = nc.declare_dram_parameter("xin", [2, C, NPAIR, PIX_CORE], BF16, isOutput=False)
    zout_d = nc.declare_dram_parameter("zout", [C, PIX_CORE], FP32, isOutput=True)

    cshapes = {
        "Wg2": ([128, 128], BF16), "bs2": ([128, 1], FP32),
        "ones_sc": ([128, 2], BF16), "Ebc": ([128, 128], BF16),
        "onesD": ([128, 64], BF16), "WuT2": ([128, 64], BF16),
        "baD": ([128, 64], BF16),
    }
    cdram = {
        k: nc.declare_dram_parameter(k, s, dt, isOutput=False)
        for k, (s, dt) in cshapes.items()
    }

    with tile.TileContext(nc) as tc:
        with (
            tc.tile_pool(name="consts", bufs=1) as cpool,
            tc.tile_pool(name="xin_p", bufs=3) as xpool,
            tc.tile_pool(name="qg_p", bufs=3) as qgpool,
            tc.tile_pool(name="esb_p", bufs=3) as epool_sb,
            tc.tile_pool(name="ew_p", bufs=4) as ewpool,
            tc.tile_pool(name="small_p", bufs=3) as smpool,
            tc.tile_pool(name="pg", bufs=2, space="PSUM") as pg,
            tc.tile_pool(name="ps", bufs=2, space="PSUM") as ps,
            tc.tile_pool(name="pe_", bufs=2, space="PSUM") as pe_,
            tc.tile_pool(name="pd", bufs=1, space="PSUM") as pd,
            tc.tile_pool(name="pu", bufs=1, space="PSUM") as pu,
        ):
            cs = {}
            for k, (s, dt) in cshapes.items():
                cs[k] = cpool.tile(s, dt, tag=f"c_{k}", name=f"c_{k}")
                if dt == BF16 or k in ("bs2", "ba"):
                    nc.sync.dma_start(out=cs[k][:], in_=cdram[k][:])
                else:
                    nc.sync.dma_start(out=_r(cs[k][:]), in_=_r(cdram[k][:]))

            # Zero both score-psum slots once: exp() later reads full tiles
            # whose unwritten rows would otherwise hold junk (NaN at boot).
            for bi in range(2):
                sz = ps.tile([128, TILE_NS[0]], FP32, tag="s_all", name=f"s_z{bi}")
                nc.vector.memset(sz[:], 0.0)

            # Warm the PE HAM clock-gate during the initial DMA window:
            # ~16 back-to-back dummy matmuls (~4us) so real matmuls start
            # at 2.4 GHz instead of the cold 1.2 GHz.
            wz = pg.tile([128, TILE_NS[0]], FP32, tag="g2", name="warm_ps")
            wrhs = bass.AP(
                tensor=cs["Wg2"].tensor, offset=cs["Wg2"].offset,
                ap=[list(cs["Wg2"].ap[0]), [0, 4], list(cs["Wg2"].ap[1])],
            )
            for _w in range(WARMUP_MMS):
                nc.tensor.matmul(wz[:], cs["Wg2"][:], wrhs, start=True, stop=True)

            n0 = 0
            for nt in TILE_NS:
                xt = xpool.tile([128, NPAIR, nt], BF16, tag="xt")
                for r in range(2):
                    nc.sync.dma_start(
                        out=xt[64 * r : 64 * r + 64, :, :],
                        in_=xin[r, :, :, n0 : n0 + nt],
                    )

                # G (replicated over both halves) = Ws * xsum + bs
                g2 = pg.tile([128, nt], FP32, tag="g2")
                for j in range(NPAIR):
                    nc.tensor.matmul(
                        g2[:], cs["Wg2"][:], xt[:, j, :],
                        start=(j == 0), stop=(j == NPAIR - 1),
                    )
                g2s = qgpool.tile([128, nt], BF16, tag="g2s")
                nc.scalar.activation(
                    out=g2s[:], in_=g2[:],
                    func=mybir.ActivationFunctionType.Identity,
                    bias=cs["bs2"][:], scale=1.0,
                )

                # score dot products: qg = G * x (bf16 out), then per-half
                # partition sums into rows 32j, 32j+1 of one psum tile
                qg = qgpool.tile([128, NPAIR, nt], BF16, tag="qg")
                for j in range(NPAIR):
                    eng = nc.gpsimd if j < QG_ON_GPSIMD else nc.vector
                    eng.tensor_mul(qg[:, j, :], g2s[:], xt[:, j, :])
                s_all = ps.tile([128, nt], FP32, tag="s_all")
                for j in range(NPAIR):
                    nc.tensor.matmul(
                        s_all[32 * j : 32 * j + 2, :], cs["ones_sc"][:],
                        qg[:, j, :],
                        start=True, stop=True, tile_position=(0, 32 * j),
                    )

                # e = exp(s) over the whole tile (junk rows are finite and
                # weighted 0 downstream)
                e_sb = epool_sb.tile([128, nt], BF16, tag="e_sb")
                nc.scalar.activation(
                    out=e_sb[:], in_=s_all[:],
                    func=mybir.ActivationFunctionType.Exp,
                )

                # weighted A sum: U = sum_m e_m * (Wa' x_m)
                u = pu.tile([64, nt], FP32, tag="u")
                nc.tensor.matmul(
                    u[:], cs["baD"][:], e_sb[:], start=True, stop=False
                )
                for j in range(NPAIR):
                    ebc_j = pe_.tile([128, nt], FP32, tag="ebc")
                    nc.tensor.matmul(
                        ebc_j[:], cs["Ebc"][32 * j : 32 * j + 2, :],
                        e_sb[32 * j : 32 * j + 2, :],
                        start=True, stop=True, tile_position=(32 * j, 0),
                    )
                    ew_j = ewpool.tile([128, nt], BF16, tag="ew")
                    nc.vector.tensor_mul(ew_j[:], ebc_j[:], xt[:, j, :])
                    nc.tensor.matmul(
                        u[:], cs["WuT2"][:], ew_j[:],
                        start=False, stop=(j == NPAIR - 1),
                    )

                # denominator and final combine
                d64 = pd.tile([64, nt], FP32, tag="d64")
                nc.tensor.matmul(
                    d64[:], cs["onesD"][:], e_sb[:], start=True, stop=True
                )
                # 1/D = exp(-ln D) on the (mostly idle) scalar engine; the
                # custom-DVE reciprocal ops don't encode on this compiler.
                lnd = smpool.tile([64, nt], FP32, tag="lnd")
                nc.scalar.activation(
                    out=lnd[:], in_=d64[:], func=mybir.ActivationFunctionType.Ln,
                )
                dinv = smpool.tile([64, nt], FP32, tag="dinv")
                nc.scalar.activation(
                    out=dinv[:], in_=lnd[:],
                    func=mybir.ActivationFunctionType.Exp, scale=-1.0,
                )
                z0 = smpool.tile([64, nt], FP32, tag="z0")
                nc.vector.tensor_mul(z0[:], u[:], dinv[:])
                nc.sync.dma_start(out=zout_d[:, n0 : n0 + nt], in_=z0[:])
                n0 += nt

    _split_multi_waits(nc)
    return nc


QG_ON_GPSIMD = 0  # first k of the 4 qg muls run on GPSIMD instead of DVE
WARMUP_MMS = 16   # dummy matmuls at start to warm the PE clock gate

_PROGRAM = None


def kernel(xs, c_w, c_b, wq_w, wq_b, wk_w, wk_b, wa_w, wa_b):
    global _PROGRAM
    xs = np.asarray(xs, dtype=np.float32)
    consts = _fuse_weights(
        np.asarray(c_w), np.asarray(c_b), np.asarray(wq_w), np.asarray(wq_b),
        np.asarray(wk_w), np.asarray(wk_b), np.asarray(wa_w), np.asarray(wa_b),
    )

    if _PROGRAM is None:
        _PROGRAM = _build_program()
    nc = _PROGRAM

    xs_bflat = xs.reshape(M, B, C, HW)
    in_maps = []
    for k in range(N_CORES):
        b = (k * PIX_CORE) // HW
        p0 = (k * PIX_CORE) % HW
        xk = xs_bflat[:, b, :, p0 : p0 + PIX_CORE]          # [M, C, PIX_CORE]
        # pair j holds m=2j (partitions 0:64) and m=2j+1 (64:128)
        x_rcjn = np.ascontiguousarray(
            xk.reshape(NPAIR, 2, C, PIX_CORE).transpose(1, 2, 0, 3)
        ).astype(ml_bf16)  # [2, C, NPAIR, PIX_CORE]
        im = {"xin": x_rcjn}
        im.update(consts)
        in_maps.append(im)

    res = bass_utils.run_bass_kernel_spmd(nc, in_maps, core_ids=list(range(N_CORES)))

    out = np.empty((B, C, HW), dtype=np.float32)
    for k in range(N_CORES):
        b = (k * PIX_CORE) // HW
        p0 = (k * PIX_CORE) % HW
        out[b, :, p0 : p0 + PIX_CORE] = res.results[k]["zout"]
    return out.reshape(B, C, H, W)


if __name__ == "__main__":
    rng = np.random.default_rng(0)
    ins = {
        "xs": rng.standard_normal((M, B, C, H, W)).astype(np.float32),
        "c_w": (rng.standard_normal((C, C)) * 0.05).astype(np.float32),
        "c_b": (rng.standard_normal((C,)) * 0.05).astype(np.float32),
        "wq_w": (rng.standard_normal((C, C)) * 0.05).astype(np.float32),
        "wq_b": (rng.standard_normal((C,)) * 0.05).astype(np.float32),
        "wk_w": (rng.standard_normal((C, C)) * 0.05).astype(np.float32),
        "wk_b": (rng.standard_normal((C,)) * 0.05).astype(np.float32),
        "wa_w": (rng.standard_normal((C, C)) * 0.05).astype(np.float32),
        "wa_b": (rng.standard_normal((C,)) * 0.05).astype(np.float32),
    }
    out = kernel(**ins)
    print("out", out.shape, out.dtype, np.abs(out).max())


# revision 26
# speedup vs baseline: 1.1304x; 1.1304x over previous
"""ATNAggregation2d Trainium2 kernel (8 NeuronCores, data-parallel over B*H*W).

Math (per pixel n, M=8 processes, C=64 channels), derived from the reference:
    V_m   = c_w x_m + c_b
    Q     = wq_w mean_m(V_m) + wq_b
    K_m   = wk_w V_m + wk_b
    A_m   = wa_w V_m + wa_b
    s_m   = (Q . K_m)/8 ;  alpha = softmax_m(s) ;  z = sum_m alpha_m A_m

Everything is linear in x, so fuse on the host:
    Wk' = wk_w c_w ; Wq' = wq_w c_w ; Wa' = wa_w c_w
    bq' = wq_w c_b + wq_b ; ba' = wa_w c_b + wa_b
    K_m's bias is constant across m -> cancels in softmax.
    s_m = Qt . (Wk' x_m)  with Qt = Q/8 = Wq' xsum/64 + bq'/8
        = G . x_m         with G = Ws xsum + bs,
          Ws = Wk'^T Wq'/64, bs = Wk'^T bq'/8
    z   = (sum_m e_m A'_m)/(sum_m e_m) + ba' , e_m = exp(s_m), A'_m = Wa' x_m
(no max-subtraction needed: |s| << 1 for these inputs/scales).

On-chip layout: C on partitions, pixels on the free dim; the 8 processes are
packed as 4 pairs stacked into 128 partitions (pair j = processes 2j, 2j+1 at
partitions 0:64 / 64:128). Per-pixel partition reductions (score dot-products,
softmax denominator, the weighted sums) are done with small matmuls.
"""

import sys

for _p in ("/opt/trn_rl_repo", "/root/.axon_site/_ro/trn_rl_repo"):
    if _p not in sys.path:
        sys.path.append(_p)

import numpy as np
from ml_dtypes import bfloat16 as ml_bf16

import concourse.bass as bass
import concourse.tile as tile
from concourse import mybir
from concourse import bass_utils

M, B, C, H, W = 8, 2, 64, 96, 96
HW = H * W
N_CORES = 8
PIX_TOTAL = B * HW                 # 18432
PIX_CORE = PIX_TOTAL // N_CORES    # 2304 contiguous pixels of flat (B, H*W)
NPAIR = M // 2                     # 4 stacked process-pairs
TILE_NS = [512, 512, 512, 512, 256]  # per-core pixel tiles (sum = 2304)

FP32 = mybir.dt.float32


def _r(ap):
    """Bitcast an fp32 AP to float32r: single-pass PE matmul (4x faster than
    the fp32 two-half-pass path) at TF32-ish multiply precision."""
    return ap.bitcast(mybir.dt.float32r)


def _split_multi_waits(nc):
    """This walrus build accepts only ONE sync-wait command per instruction.
    Move extra on_wait entries onto Drain instructions inserted just before
    the owning instruction (same engine, program order preserved)."""
    for f in nc.m.functions:
        for bb in f.blocks:
            changed = False
            new = []
            for inst in bb.instructions:
                si = inst.sync_info
                if si is not None and si.on_wait and len(si.on_wait) > 1:
                    waits = list(si.on_wait)
                    for w in waits[:-1]:
                        d = mybir.InstNoOp(
                            name=nc.get_next_instruction_name(), ins=[], outs=[]
                        )
                        d.engine = inst.engine
                        d.sync_info = mybir.SyncInfo(on_wait=[w], on_update=[])
                        new.append(d)
                    inst.sync_info = mybir.SyncInfo(
                        on_wait=[waits[-1]], on_update=list(si.on_update)
                    )
                    changed = True
                new.append(inst)
            if changed:
                bb.instructions = new


def _fuse_weights(c_w, c_b, wq_w, wq_b, wk_w, wk_b, wa_w, wa_b):
    f8 = np.float64
    c_w, c_b = c_w.astype(f8), c_b.astype(f8)
    wq_w, wq_b = wq_w.astype(f8), wq_b.astype(f8)
    wk_w = wk_w.astype(f8)
    wa_w, wa_b = wa_w.astype(f8), wa_b.astype(f8)

    Wk = wk_w @ c_w
    Wq = wq_w @ c_w
    Wa = wa_w @ c_w
    bq = wq_w @ c_b + wq_b.astype(f8)
    ba = wa_w @ c_b + wa_b

    Ws = (Wk.T @ Wq) / 64.0
    bs = (Wk.T @ bq) / 8.0

    consts = {
        # G accumulation over 4 stacked pairs: out/in both [2x64] stacked.
        "Wg2": np.tile(Ws.T, (2, 2)).astype(ml_bf16),             # [128,128]
        "bs2": np.tile(bs, 2).reshape(128, 1).astype(np.float32),  # [128,1]
        # scores: per-pair partition-half dot products -> rows 32j, 32j+1
        # (bf16 matmul: fp32r would need 64-aligned dst partitions).
        "ones_sc": np.kron(np.eye(2), np.ones((64, 16))).astype(ml_bf16),  # [128,32]
        "Ebc": None,                                               # [128,128] below
        # softmax denominator: rows 32j,32j+1 weighted 1, all others 0.
        "onesD": None,                                             # [128,64] below
        # weighted-A accumulation, folds the two stacked halves.
        "WuT2": np.tile(Wa.T, (2, 1)).astype(ml_bf16),             # [128,64]
        "baD": None,                                               # [128,64] below
    }
    # scores land as 16 replicas per process in rows 32j+16r..+16; the
    # consumers average the replicas (1/16 weights, exact in bf16).
    # Per-pair full-K broadcast planes (zero rows outside pair j's block).
    ebc4 = np.zeros((128, 4, 128), dtype=np.float32)
    for j in range(4):
        for r in range(2):
            ebc4[32 * j + 16 * r : 32 * j + 16 * r + 16, j, 64 * r : 64 * r + 64] = 1.0 / 16.0
    consts["Ebc"] = ebc4.astype(ml_bf16)                          # [128,4,128]
    consts["onesD"] = (np.ones((128, 64)) / 16.0).astype(ml_bf16)  # [128,64]
    # z = (sum_m e_m A'_m + ba * sum_m e_m)/D  ==  U/D + ba: fold the output
    # bias into the U accumulation as an extra matmul over e_sb.
    consts["baD"] = (np.tile(ba.astype(np.float32), (128, 1)) / 16.0).astype(ml_bf16)
    return consts


def _build_program(split_waits=True, debug_taps=False):
    nc = bass.Bass()
    BF16 = mybir.dt.bfloat16
    xin = nc.declare_dram_parameter("xin", [2, C, NPAIR, PIX_CORE], BF16, isOutput=False)
    zout_d = nc.declare_dram_parameter("zout", [C, PIX_CORE], FP32, isOutput=True)

    cshapes = {
        "Wg2": ([128, 128], BF16), "bs2": ([128, 1], FP32),
        "ones_sc": ([128, 32], BF16), "Ebc": ([128, 4, 128], BF16),
        "onesD": ([128, 64], BF16), "WuT2": ([128, 64], BF16),
        "baD": ([128, 64], BF16),
    }
    cdram = {
        k: nc.declare_dram_parameter(k, s, dt, isOutput=False)
        for k, (s, dt) in cshapes.items()
    }

    dbg = {}
    if debug_taps:
        BF16d = mybir.dt.bfloat16
        dbg["e_sb"] = nc.declare_dram_parameter("dbg_e", [128, 512], BF16d, isOutput=True)
        dbg["qg"] = nc.declare_dram_parameter("dbg_qg", [128, 4, 512], BF16d, isOutput=True)
        dbg["g2s"] = nc.declare_dram_parameter("dbg_g2s", [128, 512], BF16d, isOutput=True)
        dbg["ew0"] = nc.declare_dram_parameter("dbg_ew0", [128, 512], BF16d, isOutput=True)
        dbg["dinv"] = nc.declare_dram_parameter("dbg_dinv", [64, 512], mybir.dt.float32, isOutput=True)
        dbg["z0"] = nc.declare_dram_parameter("dbg_z0", [64, 512], mybir.dt.float32, isOutput=True)

    with tile.TileContext(nc) as tc:
        with (
            tc.tile_pool(name="consts", bufs=1) as cpool,
            tc.tile_pool(name="xin_p", bufs=3) as xpool,
            tc.tile_pool(name="qg_p", bufs=3) as qgpool,
            tc.tile_pool(name="esb_p", bufs=3) as epool_sb,
            tc.tile_pool(name="ew_p", bufs=2) as ewpool,
            tc.tile_pool(name="small_p", bufs=3) as smpool,
            tc.tile_pool(name="pg", bufs=2, space="PSUM") as pg,
            tc.tile_pool(name="ps", bufs=2, space="PSUM") as ps,
            tc.tile_pool(name="pe_", bufs=2, space="PSUM") as pe_,
            tc.tile_pool(name="pd", bufs=1, space="PSUM") as pd,
            tc.tile_pool(name="pu", bufs=1, space="PSUM") as pu,
        ):
            cs = {}
            for k, (s, dt) in cshapes.items():
                cs[k] = cpool.tile(s, dt, tag=f"c_{k}", name=f"c_{k}")
                if dt == BF16 or k in ("bs2", "ba"):
                    nc.sync.dma_start(out=cs[k][:], in_=cdram[k][:])
                else:
                    nc.sync.dma_start(out=_r(cs[k][:]), in_=_r(cdram[k][:]))

            # Warm the PE HAM clock-gate during the initial DMA window:
            # ~16 back-to-back dummy matmuls (~4us) so real matmuls start
            # at 2.4 GHz instead of the cold 1.2 GHz.
            wz = pg.tile([128, TILE_NS[0]], FP32, tag="g2", name="warm_ps")
            wrhs = bass.AP(
                tensor=cs["Wg2"].tensor, offset=cs["Wg2"].offset,
                ap=[list(cs["Wg2"].ap[0]), [0, 4], list(cs["Wg2"].ap[1])],
            )
            for _w in range(WARMUP_MMS):
                nc.tensor.matmul(wz[:], cs["Wg2"][:], wrhs, start=True, stop=True)

            n0 = 0
            for nt in TILE_NS:
                xt = xpool.tile([128, NPAIR, nt], BF16, tag="xt")
                for r in range(2):
                    nc.sync.dma_start(
                        out=xt[64 * r : 64 * r + 64, :, :],
                        in_=xin[r, :, :, n0 : n0 + nt],
                    )

                # G (replicated over both halves) = Ws * xsum + bs
                g2 = pg.tile([128, nt], FP32, tag="g2")
                for j in range(NPAIR):
                    nc.tensor.matmul(
                        g2[:], cs["Wg2"][:], xt[:, j, :],
                        start=(j == 0), stop=(j == NPAIR - 1),
                    )
                g2s = qgpool.tile([128, nt], BF16, tag="g2s")
                nc.scalar.activation(
                    out=g2s[:], in_=g2[:],
                    func=mybir.ActivationFunctionType.Identity,
                    bias=cs["bs2"][:], scale=1.0,
                )

                # score dot products: qg = G * x (bf16 out), then per-half
                # partition sums into rows 32j, 32j+1 of one psum tile
                qg = qgpool.tile([128, NPAIR, nt], BF16, tag="qg")
                for j in range(NPAIR):
                    nc.vector.tensor_mul(qg[:, j, :], g2s[:], xt[:, j, :])
                s_all = ps.tile([128, nt], FP32, tag="s_all")
                for j in range(NPAIR):
                    nc.tensor.matmul(
                        s_all[32 * j : 32 * j + 32, :], cs["ones_sc"][:],
                        qg[:, j, :],
                        start=True, stop=True, tile_position=(0, 32 * j),
                    )

                # e = exp(s) over the whole tile (junk rows are finite and
                # weighted 0 downstream)
                e_sb = epool_sb.tile([128, nt], BF16, tag="e_sb")
                nc.scalar.activation(
                    out=e_sb[:], in_=s_all[:],
                    func=mybir.ActivationFunctionType.Exp,
                )

                # weighted A sum: U = sum_m e_m * (Wa' x_m)
                u = pu.tile([64, nt], FP32, tag="u")
                nc.tensor.matmul(
                    u[:], cs["baD"][:], e_sb[:], start=True, stop=False
                )
                for j in range(NPAIR):
                    ebc_j = pe_.tile([128, nt], FP32, tag="ebc")
                    nc.tensor.matmul(
                        ebc_j[:], cs["Ebc"][:, j : j + 1, :], e_sb[:],
                        start=True, stop=True,
                    )
                    ew_j = ewpool.tile([128, nt], BF16, tag="ew")
                    nc.vector.tensor_mul(ew_j[:], ebc_j[:], xt[:, j, :])
                    if debug_taps and n0 == 0 and j == 0:
                        nc.sync.dma_start(out=dbg["ew0"][:], in_=ew_j[:])
                    nc.tensor.matmul(
                        u[:], cs["WuT2"][:], ew_j[:],
                        start=False, stop=(j == NPAIR - 1),
                    )

                # denominator and final combine
                d64 = pd.tile([64, nt], FP32, tag="d64")
                nc.tensor.matmul(
                    d64[:], cs["onesD"][:], e_sb[:], start=True, stop=True
                )
                # 1/D = exp(-ln D) on the (mostly idle) scalar engine; the
                # custom-DVE reciprocal ops don't encode on this compiler.
                lnd = smpool.tile([64, nt], FP32, tag="lnd")
                nc.scalar.activation(
                    out=lnd[:], in_=d64[:], func=mybir.ActivationFunctionType.Ln,
                )
                dinv = smpool.tile([64, nt], FP32, tag="dinv")
                nc.scalar.activation(
                    out=dinv[:], in_=lnd[:],
                    func=mybir.ActivationFunctionType.Exp, scale=-1.0,
                )
                z0 = smpool.tile([64, nt], FP32, tag="z0")
                nc.vector.tensor_mul(z0[:], u[:], dinv[:])
                if debug_taps and n0 == 0:
                    nc.sync.dma_start(out=dbg["dinv"][:], in_=dinv[:])
                    nc.sync.dma_start(out=dbg["z0"][:], in_=z0[:])
                    nc.sync.dma_start(out=dbg["e_sb"][:], in_=e_sb[:])
                    nc.sync.dma_start(out=dbg["qg"][:], in_=qg[:])
                    nc.sync.dma_start(out=dbg["g2s"][:], in_=g2s[:])
                nc.sync.dma_start(out=zout_d[:, n0 : n0 + nt], in_=z0[:])
                n0 += nt

    if split_waits:
        _split_multi_waits(nc)
    return nc


QG_ON_GPSIMD = 0  # first k of the 4 qg muls run on GPSIMD instead of DVE
WARMUP_MMS = 16   # dummy matmuls at start to warm the PE clock gate

_PROGRAM = None


def kernel(xs, c_w, c_b, wq_w, wq_b, wk_w, wk_b, wa_w, wa_b):
    global _PROGRAM
    xs = np.asarray(xs, dtype=np.float32)
    consts = _fuse_weights(
        np.asarray(c_w), np.asarray(c_b), np.asarray(wq_w), np.asarray(wq_b),
        np.asarray(wk_w), np.asarray(wk_b), np.asarray(wa_w), np.asarray(wa_b),
    )

    if _PROGRAM is None:
        _PROGRAM = _build_program()
    nc = _PROGRAM

    xs_bflat = xs.reshape(M, B, C, HW)
    in_maps = []
    for k in range(N_CORES):
        b = (k * PIX_CORE) // HW
        p0 = (k * PIX_CORE) % HW
        xk = xs_bflat[:, b, :, p0 : p0 + PIX_CORE]          # [M, C, PIX_CORE]
        # pair j holds m=2j (partitions 0:64) and m=2j+1 (64:128)
        x_rcjn = np.ascontiguousarray(
            xk.reshape(NPAIR, 2, C, PIX_CORE).transpose(1, 2, 0, 3)
        ).astype(ml_bf16)  # [2, C, NPAIR, PIX_CORE]
        im = {"xin": x_rcjn}
        im.update(consts)
        in_maps.append(im)

    res = bass_utils.run_bass_kernel_spmd(nc, in_maps, core_ids=list(range(N_CORES)))

    out = np.empty((B, C, HW), dtype=np.float32)
    for k in range(N_CORES):
        b = (k * PIX_CORE) // HW
        p0 = (k * PIX_CORE) % HW
        out[b, :, p0 : p0 + PIX_CORE] = res.results[k]["zout"]
    return out.reshape(B, C, H, W)


if __name__ == "__main__":
    rng = np.random.default_rng(0)
    ins = {
        "xs": rng.standard_normal((M, B, C, H, W)).astype(np.float32),
        "c_w": (rng.standard_normal((C, C)) * 0.05).astype(np.float32),
        "c_b": (rng.standard_normal((C,)) * 0.05).astype(np.float32),
        "wq_w": (rng.standard_normal((C, C)) * 0.05).astype(np.float32),
        "wq_b": (rng.standard_normal((C,)) * 0.05).astype(np.float32),
        "wk_w": (rng.standard_normal((C, C)) * 0.05).astype(np.float32),
        "wk_b": (rng.standard_normal((C,)) * 0.05).astype(np.float32),
        "wa_w": (rng.standard_normal((C, C)) * 0.05).astype(np.float32),
        "wa_b": (rng.standard_normal((C,)) * 0.05).astype(np.float32),
    }
    out = kernel(**ins)
    print("out", out.shape, out.dtype, np.abs(out).max())
